# revision 30
# baseline (speedup 1.0000x reference)
"""Trainium2 Bass kernel for AttnBlock:
GroupNorm(32) -> 1x1 q/k/v -> single-head attention over 64x64 tokens
-> 1x1 out projection -> residual.

Sharding: 8 cores = 2 batches x 4 query-chunks of 1024 tokens (token axis
rotated per core on host => pure SPMD; key order is irrelevant to GroupNorm
stats, softmax sums, and the attention contraction).

All matmuls run in fp8e4m3 with DoubleRow perf mode (256-deep contraction
per instruction, 2x bf16 throughput, fp32 PSUM accumulation). Host-side
algebraic folds shrink the graph:
  - K projection eliminated: scores = q*^T h with q* = (Wk^T Wq) h + Wk^T bq
    (the per-query offset q^T bk cancels in softmax and is dropped).
  - bv folded into beff = bo + Wo bv (sum of attn weights = 1).
  - Softmax denominator via all-ones-stationary matmul on PE (output lands
    broadcast across all partitions), applied to the attention output before
    the O projection so fp8 ranges stay O(1).
Attention-output accumulation lives in PSUM across all 16 key units per
query pass (4 banks) - no elementwise accumulation at all.
"""
import sys
sys.path.insert(0, '/opt/trn_rl_repo')
from contextlib import ExitStack

import numpy as np
import ml_dtypes
import concourse.bass as bass
import concourse.tile as tile
from concourse import bacc, mybir
from concourse.bass_utils import run_bass_kernel_spmd

F32 = mybir.dt.float32
F8 = mybir.dt.float8e4
NPF8 = ml_dtypes.float8_e4m3
DR = mybir.MatmulPerfMode.DoubleRow

C = 512
N = 4096
NQ = 1024
CT = C // 128          # 4 channel tiles
NCH = 8                # x column chunks of 512
U = 16                 # key units of 256 (2 x 128-chunks)
QH = 2                 # query passes of 512
EPS = 1e-6
SCALE = float(np.float32(int(C) ** (-0.5)))
LN16 = float(np.log(16.0))


def build(reps=1):
    nc = bacc.Bacc()
    xb = nc.dram_tensor("xb", [C, N], F32, kind="ExternalInput")
    wqk8 = nc.dram_tensor("wqk8", [C, C], F8, kind="ExternalInput")
    wv8 = nc.dram_tensor("wv8", [C, C], F8, kind="ExternalInput")
    wo8 = nc.dram_tensor("wo8", [C, C], F8, kind="ExternalInput")
    bqp = nc.dram_tensor("bqp", [C], F32, kind="ExternalInput")
    beff = nc.dram_tensor("beff", [C], F32, kind="ExternalInput")
    gamma = nc.dram_tensor("gamma", [C], F32, kind="ExternalInput")
    beta = nc.dram_tensor("beta", [C], F32, kind="ExternalInput")
    gmask = nc.dram_tensor("gmask", [128, 128], F32, kind="ExternalInput")
    out = nc.dram_tensor("out", [C, NQ], F32, kind="ExternalOutput")

    with tile.TileContext(nc) as tc, ExitStack() as ctx:
        # Pools live for the whole program. Cross-rep tiles are double
        # buffered (bufs=2) so rep n+1's x-load/GN pipeline under rep n's
        # attention passes; weights/consts load once, outside the rep loop.
        const = ctx.enter_context(tc.tile_pool(name="const", bufs=1))
        persist = ctx.enter_context(tc.tile_pool(name="persist", bufs=2))
        xqpool = ctx.enter_context(tc.tile_pool(name="xq", bufs=1))
        gtmp = ctx.enter_context(tc.tile_pool(name="gtmp", bufs=2))
        att = ctx.enter_context(tc.tile_pool(name="att", bufs=2))
        epi = ctx.enter_context(tc.tile_pool(name="epi", bufs=4))
        ps = ctx.enter_context(tc.tile_pool(name="ps", bufs=1, space="PSUM"))

        x8c0 = emit_xload(nc, 0, xb, persist, xqpool)
        # ---- rep-invariant loads ----
        def load_cvec(t, tagname):
            sb = const.tile([128, CT], F32, tag=tagname, name=tagname)
            nc.scalar.dma_start(sb[:], t[:].rearrange("(t p) -> p t", p=128))
            return sb

        gamma_sb = load_cvec(gamma, "gamma")
        beta_sb = load_cvec(beta, "beta")
        bqp_sb = load_cvec(bqp, "bqp")
        beff_sb = load_cvec(beff, "beff")
        gmask_sb = const.tile([128, 128], F32, tag="gmask")
        nc.scalar.dma_start(gmask_sb[:], gmask[:, :])
        w_sb = {}
        for wname, wdram in (("qk", wqk8), ("v", wv8), ("o", wo8)):
            wt = const.tile([128, CT, C], F8, tag=f"w{wname}", name=f"w{wname}")
            for kt in range(CT):
                nc.scalar.dma_start(wt[:, kt, :], wdram[kt * 128:(kt + 1) * 128, :])
            w_sb[wname] = wt
        eps_sb = const.tile([128, 1], F32, tag="eps")
        nc.vector.memset(eps_sb[:], EPS)
        nln16_sb = const.tile([128, 1], F32, tag="nln16")
        nc.vector.memset(nln16_sb[:], -LN16)
        ones8 = const.tile([128, 2, 128], F8, tag="ones8")
        nc.vector.memset(ones8[:], 1.0)

        emit_rep(nc, tc, 0, xb, out, persist, xqpool, gtmp, att, epi, ps,
                 gamma_sb, beta_sb, bqp_sb, beff_sb, gmask_sb, w_sb,
                 eps_sb, nln16_sb, ones8, x8c=x8c0)
        for rep in range(1, reps):
            emit_rep(nc, tc, rep, xb, out, persist, xqpool, gtmp, att, epi, ps,
                     gamma_sb, beta_sb, bqp_sb, beff_sb, gmask_sb, w_sb,
                     eps_sb, nln16_sb, ones8)

    nc.compile()
    return nc


def emit_xload(nc, rep, xb, persist, xqpool):
    """x loads: 16 DMAs on the sync+gpsimd rings ONLY (the scalar ring owns
    the output DMAs, so rep n+1's x issue never queues behind rep n's
    epilogue)."""
    R = f"r{rep}_"
    x8c = []
    for qc in range(NCH):
        xt = xqpool.tile([128, CT, 512], F32, tag=f"x{qc}", name=R + f"x{qc}")
        for hh in range(2):
            c0 = qc * 512 + hh * 256
            nc.sync.dma_start(
                xt[:, :, hh * 256:(hh + 1) * 256],
                xb[:, c0:c0 + 256].rearrange("(t p) n -> p t n", p=128))
        x8c.append(xt)
    return x8c


def emit_rep(nc, tc, rep, xb, out, persist, xqpool, gtmp, att, epi, ps,
             gamma_sb, beta_sb, bqp_sb, beff_sb, gmask_sb, w_sb,
             eps_sb, nln16_sb, ones8, x8c=None):
    R = f"r{rep}_"

    h8 = persist.tile([128, CT, N], F8, tag="h8", name=R + "h8")
    q8 = persist.tile([128, CT, NQ], F8, tag="q8", name=R + "q8")
    vt8 = persist.tile([128, N // 128, C], F8, tag="vt8", name=R + "vt8")

    if x8c is None:
        x8c = emit_xload(nc, rep, xb, persist, xqpool)

    # ---- GroupNorm stats ----
    stats = gtmp.tile([128, CT, 2 * NCH, 6], F32, tag="bst", name=R + "bst")
    for qc in range(NCH):
        for t in range(CT):
            for hh in range(2):
                nc.vector.bn_stats(
                    out=stats[:, t, qc * 2 + hh, :],
                    in_=x8c[qc][:, t, hh * 256:(hh + 1) * 256])
    stk = gtmp.tile([128, 2 * CT], F32, tag="stk", name=R + "stk")
    for t in range(CT):
        mv = gtmp.tile([128, 2], F32, tag="mv", name=R + f"mv{t}")
        nc.vector.bn_aggr(out=mv[:], in_=stats[:, t, :, :])
        nc.vector.tensor_copy(stk[:, t:t + 1], mv[:, 0:1])
        nc.vector.tensor_mul(stk[:, CT + t:CT + t + 1], mv[:, 0:1], mv[:, 0:1])
        nc.vector.tensor_add(stk[:, CT + t:CT + t + 1],
                             stk[:, CT + t:CT + t + 1], mv[:, 1:2])
    psg = ps.tile([128, 512], F32, tag="dps", name=R + "psg")
    nc.tensor.matmul(psg[:, :2 * CT], gmask_sb[:], stk[:], start=True, stop=True)
    mean_sb = gtmp.tile([128, CT], F32, tag="mean", name=R + "mean")
    nc.vector.tensor_copy(mean_sb[:], psg[:, 0:CT])
    var_sb = gtmp.tile([128, CT], F32, tag="var", name=R + "var")
    nc.vector.tensor_mul(var_sb[:], mean_sb[:], mean_sb[:])
    nc.vector.tensor_tensor(var_sb[:], psg[:, CT:2 * CT], var_sb[:],
                            mybir.AluOpType.subtract)
    nc.scalar.activation(out=var_sb[:], in_=var_sb[:],
                         func=mybir.ActivationFunctionType.Sqrt,
                         bias=eps_sb[:], scale=1.0)
    nc.vector.reciprocal(var_sb[:], var_sb[:])
    ab_sb = gtmp.tile([128, 2, CT], F32, tag="ab", name=R + "ab")
    nc.vector.tensor_mul(ab_sb[:, 0, :], var_sb[:], gamma_sb[:])
    nc.vector.tensor_mul(var_sb[:], mean_sb[:], ab_sb[:, 0, :])
    nc.vector.tensor_tensor(ab_sb[:, 1, :], beta_sb[:], var_sb[:],
                            mybir.AluOpType.subtract)

    # ---- normalize -> h8 fp8, interleaved with Q-proj and V-proj so early
    # chunks unblock downstream work; ACT keeps headroom for exp ----
    def emit_norm_chunk(qc, eng):
        for t in range(CT):
            dst = h8[:, t, qc * 512:(qc + 1) * 512]
            src = x8c[qc][:, t, :]
            if eng == 'dve':
                nc.vector.tensor_scalar(
                    out=dst, in0=src,
                    scalar1=ab_sb[:, 0, t:t + 1], scalar2=ab_sb[:, 1, t:t + 1],
                    op0=mybir.AluOpType.mult, op1=mybir.AluOpType.add)
            elif eng == 'pool':
                nc.gpsimd.tensor_scalar(
                    out=dst, in0=src,
                    scalar1=ab_sb[:, 0, t:t + 1], scalar2=ab_sb[:, 1, t:t + 1],
                    op0=mybir.AluOpType.mult, op1=mybir.AluOpType.add)
            else:
                nc.scalar.activation(
                    out=dst, in_=src,
                    func=mybir.ActivationFunctionType.Identity,
                    bias=ab_sb[:, 1, t:t + 1], scale=ab_sb[:, 0, t:t + 1])

    def emit_qproj(qh):
        qsl = slice(qh * 512, (qh + 1) * 512)
        for ct in range(CT):
            pq = ps.tile([128, 512], F32, tag="pst", name=R + f"pq{qh}{ct}", bufs=3)
            for i in range(2):
                nc.tensor.matmul(
                    pq[:], w_sb["qk"][:, 2 * i:2 * i + 2, ct * 128:(ct + 1) * 128],
                    h8[:, 2 * i:2 * i + 2, qsl],
                    start=(i == 0), stop=(i == 1), perf_mode=DR)
            nc.vector.tensor_scalar_add(out=q8[:, ct, qsl], in0=pq[:],
                                        scalar1=bqp_sb[:, ct:ct + 1])

    def emit_vproj(m, eng):
        pv = ps.tile([128, 512], F32, tag="pst", name=R + f"pv{m}", bufs=3)
        for i in range(2):
            nc.tensor.matmul(
                pv[:], h8[:, 2 * i:2 * i + 2, m * 128:(m + 1) * 128],
                w_sb["v"][:, 2 * i:2 * i + 2, :],
                start=(i == 0), stop=(i == 1), perf_mode=DR)
        if eng == 'dve':
            nc.vector.tensor_copy(vt8[:, m, :], pv[:])
        else:
            nc.scalar.activation(out=vt8[:, m, :], in_=pv[:],
                                 func=mybir.ActivationFunctionType.Copy,
                                 scale=1.0)

    emit_norm_chunk(0, 'dve')
    emit_qproj(0)
    emit_norm_chunk(1, 'dve')
    emit_qproj(1)
    norm_eng = {2: 'pool', 3: 'pool', 4: 'pool', 5: 'pool', 6: 'pool', 7: 'pool'}
    for qc in range(2, NCH):
        emit_norm_chunk(qc, norm_eng[qc])
    # V-proj for the first two chunks primes the pass; the rest is emitted
    # just-in-time inside the pass-0 unit loop so the in-order PE queue
    # never stalls on the normalize chain of late chunks
    for m in range(8):
        emit_vproj(m, 'dve' if m % 2 == 0 else 'act')

    # ---- attention passes (software-pipelined: scores of unit n+1 are
    # emitted before dsum/av of unit n so the PE never waits on exp) ----
    units = [(qh, u) for qh in range(QH) for u in range(U)]
    dps_t, pav_t, at_t = {}, {}, {}

    def emit_scores(qh, u):
        qsl = slice(qh * 512, (qh + 1) * 512)
        at = att.tile([128, 2, 512], F8, tag="at", name=R + f"at{qh}_{u}",
                      bufs=3)
        at_t[(qh, u)] = at
        for j in range(2):
            ks = u * 256 + j * 128
            pst = ps.tile([128, 512], F32, tag="pst",
                          name=R + f"pst{qh}_{u}{j}", bufs=3)
            for i in range(2):
                nc.tensor.matmul(
                    pst[:], h8[:, 2 * i:2 * i + 2, ks:ks + 128],
                    q8[:, 2 * i:2 * i + 2, qsl],
                    start=(i == 0), stop=(i == 1), perf_mode=DR)
            nc.scalar.activation(out=at[:, j, :], in_=pst[:],
                                 func=mybir.ActivationFunctionType.Exp,
                                 bias=nln16_sb[:], scale=SCALE)

    def emit_dsum_av(qh, u):
        if u == 0:
            dps_t[qh] = ps.tile([128, 512], F32, tag="dps", name=R + f"dps{qh}")
            pav_t[qh] = [ps.tile([128, 512], F32, tag=f"av{ct}",
                                 name=R + f"av{qh}{ct}") for ct in range(CT)]
        at = at_t[(qh, u)]
        nc.tensor.matmul(dps_t[qh][:], ones8[:], at[:, :, :],
                         start=(u == 0), stop=(u == U - 1), perf_mode=DR)
        for ct in range(CT):
            nc.tensor.matmul(
                pav_t[qh][ct][:], vt8[:, 2 * u:2 * u + 2, ct * 128:(ct + 1) * 128],
                at[:, :, :],
                start=(u == 0), stop=(u == U - 1), perf_mode=DR)

    def emit_attnorm(qh):
        r_bc = att.tile([128, 512], F32, tag="rbc", name=R + f"rbc{qh}", bufs=2)
        nc.vector.reciprocal(r_bc[:], dps_t[qh][:])
        av8 = att.tile([128, CT, 512], F8, tag="av8", name=R + f"av8{qh}", bufs=2)
        for ct in range(CT):
            nc.vector.tensor_tensor(av8[:, ct, :], pav_t[qh][ct][:], r_bc[:],
                                    mybir.AluOpType.mult)
        return av8

    xres_t = {}

    def emit_xres(qh):
        xres = epi.tile([128, CT, 512], F32, tag="xres", name=R + f"xres{qh}",
                        bufs=2)
        for hh in range(2):
            c0 = qh * 512 + hh * 256
            nc.gpsimd.dma_start(
                xres[:, :, hh * 256:(hh + 1) * 256],
                xb[:, c0:c0 + 256].rearrange("(t p) n -> p t n", p=128))
        xres_t[qh] = xres

    def emit_epilogue(qh, av8):
        for ct in range(CT):
            po = ps.tile([128, 512], F32, tag="pst", name=R + f"po{qh}{ct}", bufs=3)
            for i in range(2):
                nc.tensor.matmul(
                    po[:], w_sb["o"][:, 2 * i:2 * i + 2, ct * 128:(ct + 1) * 128],
                    av8[:, 2 * i:2 * i + 2, :],
                    start=(i == 0), stop=(i == 1), perf_mode=DR)
            ot = epi.tile([128, 512], F32, tag="ot", name=R + f"ot{qh}{ct}")
            for hh in range(2):
                csl = slice(hh * 256, (hh + 1) * 256)
                nc.vector.scalar_tensor_tensor(
                    out=ot[:, csl], in0=po[:, csl],
                    scalar=beff_sb[:, ct:ct + 1],
                    in1=xres_t[qh][:, ct, csl],
                    op0=mybir.AluOpType.add, op1=mybir.AluOpType.add)
                nc.gpsimd.dma_start(
                    out[ct * 128:(ct + 1) * 128,
                        qh * 512 + hh * 256:qh * 512 + (hh + 1) * 256],
                    ot[:, csl])

    emit_scores(*units[0])
    av8_0 = None
    for idx, (qh, u) in enumerate(units):
        if idx + 1 < len(units):
            emit_scores(*units[idx + 1])
        if qh == 0:
            for m in (2 * u + 8, 2 * u + 9):
                if m < N // 128:
                    emit_vproj(m, 'dve' if m % 2 == 0 else 'act')
        emit_dsum_av(qh, u)
        if (qh, u) == (0, 2):
            emit_xres(0)
        if (qh, u) == (0, U - 1):
            av8_0 = emit_attnorm(0)
        if (qh, u) == (1, 1):
            emit_epilogue(0, av8_0)
        if (qh, u) == (1, 3):
            emit_xres(1)
    av8_1 = emit_attnorm(1)
    emit_epilogue(1, av8_1)


def make_in_maps(x, gn_gamma, gn_beta, wq, bq, wk, bk, wv, bv, wo, bo):
    B = x.shape[0]
    xf = np.ascontiguousarray(np.asarray(x, np.float32).reshape(B, C, N))
    wq, wk = np.asarray(wq, np.float32), np.asarray(wk, np.float32)
    wv, wo = np.asarray(wv, np.float32), np.asarray(wo, np.float32)
    wqk = wk.T @ wq          # scores = (wqk h + wk^T bq)^T h
    base = {
        "wqk8": np.ascontiguousarray(wqk.T).astype(NPF8),
        "wv8": np.ascontiguousarray(wv.T).astype(NPF8),
        "wo8": np.ascontiguousarray(wo.T).astype(NPF8),
        "bqp": wk.T @ np.asarray(bq, np.float32),
        "beff": np.asarray(bo, np.float32) + wo @ np.asarray(bv, np.float32),
        "gamma": np.asarray(gn_gamma, np.float32),
        "beta": np.asarray(gn_beta, np.float32),
        "gmask": _gmask(),
    }
    in_maps = []
    for i in range(8):
        b, qc = i // 4, i % 4
        xrot = np.roll(xf[b], -qc * NQ, axis=1)
        in_maps.append({**base, "xb": np.ascontiguousarray(xrot)})
    return in_maps


def _gmask():
    m = np.zeros((128, 128), np.float32)
    gs = 16
    for g in range(128 // gs):
        m[g * gs:(g + 1) * gs, g * gs:(g + 1) * gs] = 1.0 / gs
    return m


def assemble(results):
    full = np.zeros((2, C, N), np.float32)
    for i in range(8):
        b, qc = i // 4, i % 4
        full[b][:, qc * NQ:(qc + 1) * NQ] = results[i]["out"]
    return full.reshape(2, C, 64, 64)


_NC_CACHE = {}


def kernel(**inputs):
    x = np.asarray(inputs["x"], np.float32)
    if "build" not in _NC_CACHE:
        _NC_CACHE["build"] = build()
    nc = _NC_CACHE["build"]
    in_maps = make_in_maps(
        x, inputs["gn_gamma"], inputs["gn_beta"],
        inputs["wq"], inputs["bq"], inputs["wk"], inputs["bk"],
        inputs["wv"], inputs["bv"], inputs["wo"], inputs["bo"])
    res = run_bass_kernel_spmd(nc, in_maps, core_ids=list(range(8)))
    return assemble(res.results)
